# revision 8
# baseline (speedup 1.0000x reference)
"""Trainium2 Bass kernel for nn_MPCActor (MLP -> condensed-QP LQR solve), v2.

Math identical to v1: condense states out of the T=5 LQR; U=[u1..u4] solves the
16x16 SPD system H U = r with H = Du + G^T diag(qx-bar) G (linear in q via the
constant P_H), u5 = -pu/qu elementwise. On device:
  - L2 phase (features on partitions, batch on free): MLP + constant linear
    maps on TensorE (f32r); ACT does all PSUM evacuations (relu/sigmoid/copies),
    transposes packed into one PSUM tile -> one evac; H|r assembled per batch
    element by TensorE and evacuated as FLOAT16.
  - L1 phase (batch on partitions+free): augmented LDL^T factorization, forward
    scale, and column-sweep back-substitution all on DVE in fp16 (2x DVE rate
    for packed 2-byte ops; reduce outputs in fp32), as a single 32-wide chain
    per group (few big instructions beat many small ones on HW), then u5 and
    f32 t-major staging for one 512B-descriptor output DMA per time step.
Sharding: pure data parallel over batch across 8 cores.
"""
import sys
import numpy as np

for _p in ("/opt/trn_rl_repo",):
    if _p not in sys.path:
        sys.path.append(_p)

import concourse.bass as bass
import concourse.mybir as mybir
import concourse.tile as tile
from concourse import bacc
from concourse.bass_utils import run_bass_kernel_spmd

S, C, OBS, T, B, HID = 12, 4, 22, 5, 65536, 512
N = S + C
nU = (T - 1) * C   # 16
nX = (T - 1) * S   # 48
NCORES = 8
BC = B // NCORES   # 8192 per core
f32 = mybir.dt.float32
f32r = mybir.dt.float32r
f16 = mybir.dt.float16
AF = mybir.ActivationFunctionType
OP = mybir.AluOpType
AX = mybir.AxisListType

E_R = nU * nU          # 256: r -> z cols 256:272
HSW = E_R + nU         # 272 = 17*16 (H | r) fp16 tile width
MW = HSW + 2 * C       # 280: + qu sel (272:276), pu sel (276:280)
nA = nU + 1


def make_consts(A, Bm):
    A = np.asarray(A, np.float64)
    Bm = np.asarray(Bm, np.float64)
    Apow = [np.eye(S)]
    for _ in range(T - 1):
        Apow.append(Apow[-1] @ A)
    G = np.zeros((nX, nU))
    Mc = np.zeros((nX, S))
    for i in range(1, T):
        Mc[(i - 1) * S:i * S] = Apow[i]
        for j in range(1, i + 1):
            G[(i - 1) * S:i * S, (j - 1) * C:j * C] = Apow[i - j] @ Bm
    Gr = G.reshape(T - 1, S, nU)
    P_H = np.einsum("tsi,tsj->ijs", Gr, Gr)
    SG = np.einsum("tsi->si", Gr)
    MHF = np.zeros((2 * N, MW))
    for i in range(nU):
        for j in range(nU):
            MHF[:S, i * nU + j] = P_H[i, j]
        MHF[S + i % C, i * nU + i] += 1.0
    # r linear-in-y part (cols 256:272)
    MHF[N:N + S, E_R:HSW] = -SG
    for i in range(nU):
        MHF[N + S + i % C, E_R + i] += -1.0
    # qu / pu selectors (cols 272:280)
    for c in range(C):
        MHF[S + c, HSW + c] = 1.0
        MHF[N + S + c, HSW + C + c] = 1.0
    Mqbar = np.zeros((2 * N, nX))
    for t in range(T - 1):
        for s in range(S):
            Mqbar[s, t * S + s] = 1.0
    z = np.float32
    return dict(MHF=MHF.astype(z), Mqbar=Mqbar.astype(z),
                McT=np.ascontiguousarray(Mc.T, z), Gneg=(-G).astype(z))


def build(bc=BC, repeat=1, ngrp=2, nview=1):
    """Build the per-core SPMD program. bc = per-core batch."""
    nb = 512
    nchunk = bc // nb
    assert nchunk % ngrp == 0
    gch = nchunk // ngrp
    grp_start = [g * gch for g in range(ngrp)]
    bsub = 4 * gch
    assert bsub % nview == 0

    nc = bacc.Bacc("TRN2", target_bir_lowering=False, debug=False)

    obs_d = nc.declare_dram_parameter("obs", [bc, OBS], f32r, isOutput=False)
    x1_d = nc.declare_dram_parameter("x_init", [bc, S], f32r, isOutput=False)
    W1_d = nc.declare_dram_parameter("W1", [OBS, HID], f32r, isOutput=False)
    b1_d = nc.declare_dram_parameter("b1", [HID], f32, isOutput=False)
    W2_d = nc.declare_dram_parameter("W2", [HID, HID], f32r, isOutput=False)
    b2_d = nc.declare_dram_parameter("b2", [HID], f32, isOutput=False)
    W3_d = nc.declare_dram_parameter("W3", [HID, 2 * N], f32r, isOutput=False)
    b3_d = nc.declare_dram_parameter("b3", [2 * N], f32, isOutput=False)
    MHF_d = nc.declare_dram_parameter("MHF", [2 * N, MW], f32r, isOutput=False)
    Mqbar_d = nc.declare_dram_parameter("Mqbar", [2 * N, nX], f32r, isOutput=False)
    McT_d = nc.declare_dram_parameter("McT", [S, nX], f32r, isOutput=False)
    Gneg_d = nc.declare_dram_parameter("Gneg", [nX, nU], f32r, isOutput=False)
    id_d = nc.declare_dram_parameter("ident", [128, 128], f32r, isOutput=False)
    u_d = nc.declare_dram_parameter("u", [T, bc, C], f32, isOutput=True)

    obs_v = obs_d.ap().rearrange("(p i) f -> p i f", i=bc // 128)
    x1_v = x1_d.ap().rearrange("(p i) f -> p i f", i=bc // 128)
    u_v = u_d.ap().rearrange("t (p i) c -> t p i c", i=bc // 128)

    def r32(ap):
        return ap.bitcast(f32r)

    with tile.TileContext(nc) as tc:
        with tc.tile_pool(name="const", bufs=1) as cp, \
             tc.tile_pool(name="work", bufs=4) as wp, \
             tc.tile_pool(name="hs", bufs=1) as hp, \
             tc.tile_pool(name="slv", bufs=1) as sp, \
             tc.tile_pool(name="psmm", bufs=4, space="PSUM") as pmm, \
             tc.tile_pool(name="pstp", bufs=1, space="PSUM") as ptp:

            # ---- constants ----
            ident = cp.tile([128, 128], f32r, tag="ident")
            nc.sync.dma_start(out=ident, in_=id_d.ap())
            w1sb = cp.tile([OBS, HID], f32r, tag="w1")
            nc.sync.dma_start(out=w1sb, in_=W1_d.ap())
            w2sb = []
            for k in range(4):
                t_ = cp.tile([128, HID], f32r, tag=f"w2_{k}")
                nc.sync.dma_start(out=t_, in_=W2_d.ap()[128 * k:128 * (k + 1), :])
                w2sb.append(t_)
            w3sb = []
            for k in range(4):
                t_ = cp.tile([128, 2 * N], f32r, tag=f"w3_{k}")
                nc.sync.dma_start(out=t_, in_=W3_d.ap()[128 * k:128 * (k + 1), :])
                w3sb.append(t_)
            b1sb = cp.tile([128, 4], f32, tag="b1")
            nc.sync.dma_start(out=b1sb, in_=b1_d.ap().rearrange("(m p) -> p m", p=128))
            b2sb = cp.tile([128, 4], f32, tag="b2")
            nc.sync.dma_start(out=b2sb, in_=b2_d.ap().rearrange("(m p) -> p m", p=128))
            b3sb = cp.tile([2 * N, 1], f32, tag="b3")
            nc.sync.dma_start(out=b3sb, in_=b3_d.ap().rearrange("(m o) -> m o", o=1))
            mhf = cp.tile([2 * N, MW], f32r, tag="mhf")
            nc.sync.dma_start(out=mhf, in_=MHF_d.ap())
            mqbar = cp.tile([2 * N, nX], f32r, tag="mqbar")
            nc.sync.dma_start(out=mqbar, in_=Mqbar_d.ap())
            mct = cp.tile([32 + S, nX], f32r, tag="mct")
            nc.sync.dma_start(out=mct[32:32 + S, :], in_=McT_d.ap())
            gneg = cp.tile([nX, nU], f32r, tag="gneg")
            nc.sync.dma_start(out=gneg, in_=Gneg_d.ap())

            hsH_t = [hp.tile([128, bsub, MW], f16, tag=f"hsH{g}", name=f"hsH{g}")
                     for g in range(ngrp)]

            def chunk_phase(ch, g):
                il0 = 4 * (ch - grp_start[g])
                hsH = hsH_t[g]

                # obs in cols 0:22, x1 in cols 32:44 of one tile; pad 22:32
                # memset so the fused transpose reads initialized data
                ox_c = wp.tile([128, 4, 32 + S], f32r, tag="ox_c")
                nc.vector.memset(ox_c[:, :, OBS:32].bitcast(f32), 0.0)
                nc.sync.dma_start(out=ox_c[:, :, 0:OBS],
                                  in_=obs_v[:, 4 * ch:4 * ch + 4, :])
                nc.sync.dma_start(out=ox_c[:, :, 32:32 + S],
                                  in_=x1_v[:, 4 * ch:4 * ch + 4, :])

                # one fused transpose per 128-block: rows 0:22 obs^T, 32:44 x1^T
                tp = pmm.tile([32 + S, nb], f32, tag="mm")
                for t in range(4):
                    nc.tensor.transpose(out=r32(tp[:, 128 * t:128 * (t + 1)]),
                                        in_=ox_c[:, t, :], identity=ident)
                obsx = wp.tile([32 + S, nb], f32r, tag="obsx")
                nc.scalar.copy(out=obsx, in_=tp)

                # layer 1
                h1sb = []
                for mc in range(4):
                    ps = pmm.tile([128, nb], f32, tag="mm")
                    nc.tensor.matmul(out=ps, lhsT=w1sb[:, 128 * mc:128 * (mc + 1)],
                                     rhs=obsx[0:OBS, :], start=True, stop=True)
                    hsb = wp.tile([128, nb], f32r, tag=f"h1_{mc}")
                    nc.scalar.activation(out=hsb, in_=ps, func=AF.Relu,
                                         bias=b1sb[:, mc:mc + 1], scale=1.0)
                    h1sb.append(hsb)
                # layer 2
                h2sb = []
                for mc in range(4):
                    ps = pmm.tile([128, nb], f32, tag="mm")
                    for kc in range(4):
                        nc.tensor.matmul(out=ps,
                                         lhsT=w2sb[kc][:, 128 * mc:128 * (mc + 1)],
                                         rhs=h1sb[kc],
                                         start=(kc == 0), stop=(kc == 3))
                    hsb = wp.tile([128, nb], f32r, tag=f"h2_{mc}")
                    nc.scalar.activation(out=hsb, in_=ps, func=AF.Relu,
                                         bias=b2sb[:, mc:mc + 1], scale=1.0)
                    h2sb.append(hsb)
                # layer 3 + sigmoid
                ps_y = pmm.tile([2 * N, nb], f32, tag="mm")
                for kc in range(4):
                    nc.tensor.matmul(out=ps_y, lhsT=w3sb[kc], rhs=h2sb[kc],
                                     start=(kc == 0), stop=(kc == 3))
                ysb = wp.tile([2 * N, nb], f32r, tag="ysb")
                nc.scalar.activation(out=ysb, in_=ps_y, func=AF.Sigmoid,
                                     bias=b3sb[:, 0:1], scale=1.0)

                # c = Mc x1 ; qxbar = Mqbar y ; prod = c * qxbar
                ps_c = pmm.tile([nX, nb], f32, tag="mm")
                nc.tensor.matmul(out=ps_c, lhsT=mct[32:32 + S, :],
                                 rhs=obsx[32:32 + S, :], start=True, stop=True)
                c_sb = wp.tile([nX, nb], f32r, tag="c_sb")
                nc.scalar.copy(out=c_sb, in_=ps_c)
                ps_qb = pmm.tile([nX, nb], f32, tag="mm")
                nc.tensor.matmul(out=ps_qb, lhsT=mqbar, rhs=ysb,
                                 start=True, stop=True)
                prod = wp.tile([nX, nb], f32r, tag="prod")
                nc.vector.tensor_mul(out=prod, in0=c_sb, in1=ps_qb)

                # [H | r | qu | pu] per batch element
                ps_h = ptp.tile([128, 4, 512], f32, tag="tps")
                for b4 in range(4):
                    blk = slice(128 * b4, 128 * (b4 + 1))
                    nc.tensor.matmul(out=ps_h[:, b4, 0:MW], lhsT=ysb[:, blk],
                                     rhs=mhf, start=True, stop=False)
                for b4 in range(4):
                    blk = slice(128 * b4, 128 * (b4 + 1))
                    nc.tensor.matmul(out=ps_h[:, b4, E_R:HSW], lhsT=prod[:, blk],
                                     rhs=gneg, start=False, stop=True)
                nc.scalar.copy(out=hsH[:, il0:il0 + 4, :], in_=ps_h[:, :, 0:MW])

            def solve_group(g):
                with nc.allow_low_precision(reason="fp16 LDL validated offline"):
                    _solve_group(g)

            def _solve_group(g):
                hsH = hsH_t[g]
                Hv_f = hsH[:, :, 0:HSW].rearrange("p i (a b) -> p i a b", b=nU)
                zv_f = hsH[:, :, E_R:E_R + nU]
                rd = sp.tile([128, bsub, nU], f16, tag=f"rd{g}", name=f"rd{g}")
                fk = sp.tile([128, bsub, nU], f16, tag=f"fk{g}", name=f"fk{g}")
                tmp = sp.tile([128, bsub, 80], f16, tag=f"tmp{g}", name=f"tmp{g}")
                red = sp.tile([128, bsub, nU], f32, tag=f"red{g}", name=f"red{g}")
                zf32 = sp.tile([128, bsub, nU + C], f32, tag=f"zf{g}", name=f"zf{g}")

                vs = bsub // nview
                views = []
                for v in range(nview):
                    sl = slice(v * vs, (v + 1) * vs)
                    views.append(dict(
                        n=vs, Hv=Hv_f[:, sl], zv=zv_f[:, sl], rd=rd[:, sl],
                        fk=fk[:, sl], tmp=tmp[:, sl], red=red[:, sl]))

                # augmented LDL^T: column j updates rows j..16 (row 16 = r)
                for j in range(nU):
                    for V in views:
                        Hv, rdv, fkv, redv, tmpv, n_i = (
                            V["Hv"], V["rd"], V["fk"], V["red"], V["tmp"], V["n"])
                        m = nA - j
                        if j == 1:
                            nc.vector.tensor_mul(out=fkv[:, :, :1],
                                                 in0=Hv[:, :, 1, :1],
                                                 in1=rdv[:, :, :1])
                            nc.vector.tensor_mul(
                                out=redv[:, :, :m], in0=Hv[:, :, 1:, 0],
                                in1=fkv[:, :, 0:1].broadcast_to([128, n_i, m]))
                            nc.vector.tensor_sub(out=Hv[:, :, j:, j],
                                                 in0=Hv[:, :, j:, j],
                                                 in1=redv[:, :, :m])
                        elif j > 1:
                            nc.vector.tensor_mul(out=fkv[:, :, :j],
                                                 in0=Hv[:, :, j, :j],
                                                 in1=rdv[:, :, :j])
                            tv = tmpv[:, :, :m * j].rearrange(
                                "p i (a k) -> p i a k", k=j)
                            nc.vector.tensor_mul(
                                out=tv, in0=Hv[:, :, j:, :j],
                                in1=fkv[:, :, :j].unsqueeze(2).broadcast_to(
                                    [128, n_i, m, j]))
                            nc.vector.tensor_reduce(out=redv[:, :, :m], in_=tv,
                                                    axis=AX.X, op=OP.add)
                            nc.vector.tensor_sub(out=Hv[:, :, j:, j],
                                                 in0=Hv[:, :, j:, j],
                                                 in1=redv[:, :, :m])
                        nc.vector.reciprocal(out=rdv[:, :, j:j + 1],
                                             in_=Hv[:, :, j, j])
                # forward scale + back-substitution on Pool (GPSIMD), full bsub
                # z = w = (L^-1 r) * rd (row 16 of Hv, in place)
                nc.vector.tensor_mul(out=zv_f, in0=Hv_f[:, :, nU, :], in1=rd)
                # column sweep: z[:k] -= (H[k,:k]*rd[:k]) * z[k]
                for k in range(nU - 1, 0, -1):
                    nc.vector.tensor_mul(out=fk[:, :, :k], in0=Hv_f[:, :, k, :k],
                                         in1=rd[:, :, :k])
                    nc.vector.tensor_mul(
                        out=tmp[:, :, :k], in0=fk[:, :, :k],
                        in1=zv_f[:, :, k:k + 1].broadcast_to([128, bsub, k]))
                    nc.vector.tensor_sub(out=zv_f[:, :, :k], in0=zv_f[:, :, :k],
                                         in1=tmp[:, :, :k])
                # u5 = -pu * (1/qu): DVE reciprocal, Pool multiply + negate
                zf4 = zf32.rearrange("p i (t c) -> p t i c", c=C)
                rq = red[:, :, 0:C]
                nc.vector.reciprocal(out=rq, in_=hsH[:, :, HSW:HSW + C])
                nc.vector.tensor_mul(out=zf4[:, T - 1, :, :], in0=rq,
                                     in1=hsH[:, :, HSW + C:HSW + 2 * C])
                nc.vector.tensor_scalar_mul(zf4[:, T - 1, :, :],
                                            zf4[:, T - 1, :, :], -1.0)
                nc.vector.tensor_copy(
                    out=zf4[:, 0:T - 1, :, :],
                    in_=zv_f.rearrange("p i (t c) -> p t i c", c=C))
                # output DMAs: i rows contiguous per (t,p) -> 512B descriptors
                i0 = 4 * grp_start[g]
                for t in range(T):
                    nc.sync.dma_start(out=u_v[t, :, i0:i0 + bsub, :],
                                      in_=zf4[:, t, :, :])

            for _rep in range(repeat):
                for g in range(ngrp):
                    for ch in range(grp_start[g], grp_start[g] + gch):
                        chunk_phase(ch, g)
                    solve_group(g)

    nc.compile()
    return nc


_NC_CACHE = {}


def _get_nc(bc):
    if bc not in _NC_CACHE:
        _NC_CACHE[bc] = build(bc)
    return _NC_CACHE[bc]


def kernel(obs, x_init, W1, b1, W2, b2, W3, b3, A, Bm):
    obs = np.ascontiguousarray(obs, np.float32)
    x_init = np.ascontiguousarray(x_init, np.float32)
    cst = make_consts(A, Bm)
    nc = _get_nc(BC)
    shared = dict(W1=np.ascontiguousarray(W1, np.float32),
                  b1=np.ascontiguousarray(b1, np.float32),
                  W2=np.ascontiguousarray(W2, np.float32),
                  b2=np.ascontiguousarray(b2, np.float32),
                  W3=np.ascontiguousarray(W3, np.float32),
                  b3=np.ascontiguousarray(b3, np.float32),
                  ident=np.eye(128, dtype=np.float32), **cst)
    in_maps = []
    for k in range(NCORES):
        sl = slice(k * BC, (k + 1) * BC)
        in_maps.append(dict(obs=obs[sl], x_init=x_init[sl], **shared))
    res = run_bass_kernel_spmd(nc, in_maps, list(range(NCORES)))
    out = np.empty((T, B, C), np.float32)
    for k in range(NCORES):
        out[:, k * BC:(k + 1) * BC, :] = res.results[k]["u"]
    return out
